# revision 12
# baseline (speedup 1.0000x reference)
"""Trainium2 Bass kernel for nn_AttentionLayer (B=4, T=2048, C=1024, H=16).

Sharding (8 cores): core c = (batch b = c//2, head-group g = c%2).
Data parallel on batch, tensor parallel on heads: each core computes the
qkv projection for its 8 heads, causal flash-attention, and a partial
output projection (row split of w_proj). Host sums the two partials per
batch and re-transposes.

Per-core kernel (Bass/Tile, fp32r matmuls = TF32-like fp22 PE mode):
  phase A: qkv projection.  Q^T/K^T produced in [head_dim, t] layout
           (moving operand = x^T), V in natural [t, head_dim] layout
           (moving operand = w_v^T) with an appended ones column.
  phase B: causal attention per head-pair.  S^T = K^T.T @ Q^T row-tiled
           2 heads/matmul (contraction 64 x 2), exp on ACT (no
           max-subtract needed: logits are O(1)), causal mask via
           gpsimd.affine_select on diagonal tiles, O^T = [V|1].T @ P^T
           accumulated in PSUM; row 64 gives softmax denominators;
           normalize via reciprocal + gpsimd partition_broadcast.
  phase C: out^T = w_p^T.T @ y^T + bias (bias only on g=0 cores).

All DRAM tensors are host-pre-tiled so every DMA is one contiguous block.
"""
from contextlib import ExitStack

import numpy as np

import concourse.bacc as bacc
import concourse.mybir as mybir
import concourse.tile as tile
from concourse.bass_utils import run_bass_kernel_spmd

F32 = mybir.dt.float32
F32R = mybir.dt.float32r
AF = mybir.ActivationFunctionType

B, T, C, H = 4, 2048, 1024, 16
HD = C // H          # 64
NH = H // 2          # heads per core: 8
QCOLS = NH * HD      # 512


def build(T=T, C=C, NH=NH, HD=HD, TQ=512, loop_iters=1):
    assert C % 128 == 0 and T % TQ == 0 and TQ % 128 == 0
    NP = NH // 2              # head pairs
    CT = C // 128             # contraction tiles
    NTB = T // TQ             # time blocks
    TT = T // 128             # tk tiles
    NO = C // 128             # out row tiles
    QC = NH * HD
    scale = 1.0 / (HD ** 0.5)

    nc = bacc.Bacc()
    xT = nc.declare_dram_parameter("xT", [CT, NTB, 128, TQ], F32R, isOutput=False)
    wqkT = nc.declare_dram_parameter("wqkT", [C, 2 * QC], F32R, isOutput=False)
    wvT = nc.declare_dram_parameter("wvT", [CT, 128, QC], F32R, isOutput=False)
    wpT = nc.declare_dram_parameter("wpT", [NP, 128, C], F32R, isOutput=False)
    bias = nc.declare_dram_parameter("bias", [128, NO], F32, isOutput=False)
    outT = nc.declare_dram_parameter("outT", [NO, NTB, 128, TQ], F32, isOutput=True)

    with tile.TileContext(nc) as tc, ExitStack() as ctx:
        # long-lived pools first (stack allocator)
        qt_pool = ctx.enter_context(tc.tile_pool(name="qt", bufs=NP * NTB))
        kt_pool = ctx.enter_context(tc.tile_pool(name="kt", bufs=NP * NTB))
        v_pool = ctx.enter_context(tc.tile_pool(name="v", bufs=TT))
        wp_pool = ctx.enter_context(tc.tile_pool(name="wp", bufs=NP))
        bias_pool = ctx.enter_context(tc.tile_pool(name="bias", bufs=1))

        bias_sb = bias_pool.tile([128, NO], F32, tag="bias", name="bias_sb")
        nc.sync.dma_start(bias_sb[:], bias[:])
        ones_sb = bias_pool.tile([128, NH], F32, tag="ones", name="ones_sb")
        nc.gpsimd.memset(ones_sb[:], 1.0)
        wp_sb = [wp_pool.tile([128, C], F32R, tag="wp", name="wp") for p in range(NP)]
        for p in range(NP):
            nc.sync.dma_start(wp_sb[p][:], wpT[p])

        qt = {}
        kt = {}
        vt = []
        yt = {}

        def body():
            qt.clear(); kt.clear(); vt.clear(); yt.clear()
            ctx2 = ExitStack()
            st_pool = ctx2.enter_context(tc.tile_pool(name="st", bufs=2, space="PSUM"))
            o_ps_pool = ctx2.enter_context(tc.tile_pool(name="ops", bufs=2, space="PSUM"))
            pt_pool = ctx2.enter_context(tc.tile_pool(name="pt", bufs=4))
            rc_pool = ctx2.enter_context(tc.tile_pool(name="rc", bufs=4))
            osb_pool = ctx2.enter_context(tc.tile_pool(name="osb", bufs=3))

            def emit_proj_block(tb, wqk_pool, xs_pool, mm_ps, wvs):
                """Phase A chunk: Q^T/K^T j-tiles + V tiles for one time block."""
                xs = [xs_pool.tile([128, TQ], F32R, tag="xs", name="xs") for _ in range(CT)]
                for c in range(CT):
                    nc.sync.dma_start(xs[c][:], xT[c, tb])
                # half 0 = Q cols, half 1 = K cols of wqkT
                for half in range(2):
                    ws = []
                    for c in range(CT):
                        w = wqk_pool.tile([128, QC], F32R, tag="wqk", name="wqk")
                        nc.sync.dma_start(w[:], wqkT[128 * c:128 * (c + 1),
                                                     half * QC:(half + 1) * QC])
                        ws.append(w)
                    for jp in range(NP):
                        jt = half * NP + jp
                        ps = mm_ps.tile([128, TQ], F32, tag="mm", name="mm")
                        for c in range(CT):
                            nc.tensor.matmul(ps[:], ws[c][:, 128 * jp:128 * (jp + 1)], xs[c][:],
                                             start=(c == 0), stop=(c == CT - 1))
                        dst = qt_pool.tile([128, TQ], F32R, tag="qt", name="qt") if jt < NP else kt_pool.tile([128, TQ], F32R, tag="kt", name="kt")
                        nc.vector.tensor_copy(dst[:], ps[:])
                        if jt < NP:
                            qt[(jt, tb)] = dst
                        else:
                            kt[(jt - NP, tb)] = dst
                for ti in range(TQ // 128):
                    tt_i = tb * (TQ // 128) + ti
                    ps = mm_ps.tile([128, QC], F32, tag="mm", name="mmv")
                    for c in range(CT):
                        nc.tensor.matmul(ps[:], xs[c][:, 128 * ti:128 * (ti + 1)], wvs[c][:],
                                         start=(c == 0), stop=(c == CT - 1))
                    vtile = v_pool.tile([128, NH * (HD + 1)], F32R, tag="v", name="v")
                    v3 = vtile[:].rearrange("p (h d) -> p h d", d=HD + 1)
                    nc.vector.tensor_copy(v3[:, :, 0:HD], ps[:].rearrange("p (h d) -> p h d", d=HD))
                    nc.vector.tensor_copy(v3[:, :, HD], ones_sb[:])
                    assert len(vt) == tt_i
                    vt.append(vtile)

            def emit_attention_block(qi):
                """Phase B chunk (all pairs, one query block) + phase C for it."""
                tq0 = qi * TQ
                ntk = (tq0 + TQ) // 128
                for p in range(NP):
                    h0 = 2 * p
                    h1 = 2 * p + 1
                    o0 = o_ps_pool.tile([HD + 1, TQ], F32, tag="ops", name="ops")
                    o1 = o_ps_pool.tile([HD + 1, TQ], F32, tag="ops", name="ops2")
                    for tki in range(ntk):
                        tk0 = tki * 128
                        # diagonal narrowing: only q positions >= tk0 can
                        # attend; min width 256 (f32r matmul slows below)
                        dlt = min(max(0, tk0 - tq0), TQ - 256)
                        w = TQ - dlt
                        diag = tk0 >= tq0
                        ktile = kt[(p, tk0 // TQ)]
                        koff = tk0 % TQ
                        qtile = qt[(p, qi)]
                        st = st_pool.tile([128, 2 * TQ], F32, tag="st", name="st")
                        nc.tensor.matmul(st[:, 0:w], ktile[0:64, koff:koff + 128],
                                         qtile[0:64, dlt:TQ], start=True, stop=True)
                        nc.tensor.matmul(st[:, TQ:TQ + w], ktile[64:128, koff:koff + 128],
                                         qtile[64:128, dlt:TQ], start=True, stop=True)
                        pt = pt_pool.tile([128, 2 * TQ], F32R, tag="pt", name="pt")
                        st_v = st[:].rearrange("p (h q) -> p h q", q=TQ)[:, :, 0:w]
                        pt_v = pt[:, 0:2 * w].rearrange("p (h q) -> p h q", h=2)
                        nc.scalar.activation(pt_v, st_v, AF.Exp, scale=scale)
                        if diag:
                            bw = min(128 + (tk0 - tq0) - dlt, w)
                            base = -((tk0 - tq0) - dlt)
                            for half in range(2):
                                nc.gpsimd.affine_select(
                                    out=pt[:, half * w:half * w + bw],
                                    in_=pt[:, half * w:half * w + bw],
                                    compare_op=mybir.AluOpType.is_ge,
                                    fill=0.0, base=base,
                                    pattern=[[1, bw]], channel_multiplier=-1)
                        vtile = vt[tki]
                        v3 = vtile[:].rearrange("p (h d) -> p h d", d=HD + 1)
                        nc.tensor.matmul(o0[:, dlt:TQ], v3[:, h0, :], pt[:, 0:w],
                                         start=(tki == 0), stop=(tki == ntk - 1))
                        nc.tensor.matmul(o1[:, dlt:TQ], v3[:, h1, :], pt[:, w:2 * w],
                                         start=(tki == 0), stop=(tki == ntk - 1))
                    ytile = qt_pool.tile([128, TQ], F32R, tag="qt", name="y")
                    yt[(p, qi)] = ytile
                    for h, ops in ((0, o0), (1, o1)):
                        rc = rc_pool.tile([1, TQ], F32, tag="rc", name="rc")
                        nc.vector.reciprocal(rc[:], ops[HD:HD + 1, :])
                        bc = rc_pool.tile([HD, TQ], F32, tag="bc", name="bc")
                        nc.gpsimd.partition_broadcast(bc[:], rc[:])
                        nc.vector.tensor_mul(ytile[64 * h:64 * h + 64, :], ops[0:HD, :], bc[:])
                # phase C for this time block
                tb = qi
                for ot in range(NO):
                    ps = o_ps_pool.tile([128, TQ], F32, tag="ops", name="mmo")
                    for p in range(NP):
                        nc.tensor.matmul(ps[:], wp_sb[p][:, 128 * ot:128 * (ot + 1)], yt[(p, tb)][:],
                                         start=(p == 0), stop=(p == NP - 1))
                    osb = osb_pool.tile([128, TQ], F32, tag="osb", name="osb")
                    nc.vector.tensor_scalar_add(osb[:], ps[:], bias_sb[:, ot:ot + 1])
                    nc.sync.dma_start(outT[ot, tb], osb[:])

            # interleave: A(tb) ... B(qi=tb-1) so exp/attention overlaps the
            # next projection block on ACT/Pool/DVE while PE streams on.
            with tc.tile_pool(name="wv_s", bufs=CT) as wv_pool, \
                 tc.tile_pool(name="wqk_s", bufs=CT + 2) as wqk_pool, \
                 tc.tile_pool(name="xs", bufs=CT + 1) as xs_pool, \
                 tc.tile_pool(name="mmA", bufs=2, space="PSUM") as mm_ps:
                wvs = []
                for c in range(CT):
                    w = wv_pool.tile([128, QC], F32R, tag="wv", name="wv")
                    nc.sync.dma_start(w[:], wvT[c])
                    wvs.append(w)
                for tb in range(NTB):
                    emit_proj_block(tb, wqk_pool, xs_pool, mm_ps, wvs)
                    if tb >= 1:
                        emit_attention_block(tb - 1)
            emit_attention_block(NTB - 1)
            ctx2.close()

        if loop_iters == 1:
            body()
        else:
            with tc.For_i(0, loop_iters, 1):
                body()
    nc.finalize()
    return nc


def _tile2d(a, pr, pc):
    """[R, S] -> [R//pr, S//pc, pr, pc] contiguous tiles."""
    R, S = a.shape
    return np.ascontiguousarray(
        a.reshape(R // pr, pr, S // pc, pc).transpose(0, 2, 1, 3))


def shard_inputs(x, w_attn, w_proj, b_proj, TQ=512):
    """Returns in_maps for 8 cores: core c = (b=c//2, g=c%2)."""
    CT = C // 128
    NP = NH // 2
    NTB = T // TQ
    wq, wk, wv = w_attn[0:C], w_attn[C:2 * C], w_attn[2 * C:3 * C]
    x = np.asarray(x)
    in_maps = []
    for core in range(8):
        b = core // 2
        g = core % 2
        rows = slice(g * QCOLS, (g + 1) * QCOLS)
        xTt = _tile2d(np.asarray(x[b]).T, 128, TQ)                       # [CT,NTB,128,TQ]
        wqkTt = np.ascontiguousarray(np.concatenate([wq[rows], wk[rows]], 0).T)  # [C,2QC]
        wvTt = np.ascontiguousarray(wv[rows].T.reshape(CT, 128, QCOLS))
        wpTt = np.ascontiguousarray(w_proj[:, rows].T.reshape(NP, 128, C))
        in_maps.append({
            "xT": xTt,
            "wqkT": wqkTt,
            "wvT": wvTt,
            "wpT": wpTt,
            "bias": (np.ascontiguousarray(b_proj.reshape(C // 128, 128).T)
                     if g == 0 else np.zeros((128, C // 128), np.float32)),
        })
    return in_maps


def unshard_output(outT_tiles_pair, TQ=512):
    """outT [NO,NTB,128,TQ] partials (2 cores) -> out [T, C]."""
    s = outT_tiles_pair[0] + outT_tiles_pair[1]
    NO, NTB = C // 128, T // TQ
    return s.transpose(0, 2, 1, 3).reshape(C, T).T


_NC_CACHE = {}


def kernel(x, w_attn, w_proj, b_proj):
    if "nc" not in _NC_CACHE:
        _NC_CACHE["nc"] = build()
    nc = _NC_CACHE["nc"]
    in_maps = shard_inputs(x, w_attn, w_proj, b_proj)
    res = run_bass_kernel_spmd(nc, in_maps, core_ids=list(range(8)))
    out = np.empty((B, T, C), np.float32)
    for b in range(B):
        out[b] = unshard_output([res.results[2 * b]["outT"],
                                 res.results[2 * b + 1]["outT"]])
    return out


# revision 13
# speedup vs baseline: 1.0766x; 1.0766x over previous
"""Trainium2 Bass kernel for nn_AttentionLayer (B=4, T=2048, C=1024, H=16).

Sharding (8 cores): core c = (batch b = c//2, head-group g = c%2).
Data parallel on batch, tensor parallel on heads: each core computes the
qkv projection for its 8 heads, causal flash-attention, and a partial
output projection (row split of w_proj). Host sums the two partials per
batch and re-transposes.

Per-core kernel (Bass/Tile, fp32r matmuls = TF32-like fp22 PE mode):
  phase A: qkv projection.  Q^T/K^T produced in [head_dim, t] layout
           (moving operand = x^T), V in natural [t, head_dim] layout
           (moving operand = w_v^T) with an appended ones column.
  phase B: causal attention per head-pair.  S^T = K^T.T @ Q^T row-tiled
           2 heads/matmul (contraction 64 x 2), exp on ACT (no
           max-subtract needed: logits are O(1)), causal mask via
           gpsimd.affine_select on diagonal tiles, O^T = [V|1].T @ P^T
           accumulated in PSUM; row 64 gives softmax denominators;
           normalize via reciprocal + gpsimd partition_broadcast.
  phase C: out^T = w_p^T.T @ y^T + bias (bias only on g=0 cores).

All DRAM tensors are host-pre-tiled so every DMA is one contiguous block.
"""
from contextlib import ExitStack

import numpy as np

import concourse.bacc as bacc
import concourse.mybir as mybir
import concourse.tile as tile
from concourse.bass_utils import run_bass_kernel_spmd

F32 = mybir.dt.float32
F32R = mybir.dt.float32r
AF = mybir.ActivationFunctionType

B, T, C, H = 4, 2048, 1024, 16
HD = C // H          # 64
NH = H // 2          # heads per core: 8
QCOLS = NH * HD      # 512


def build(T=T, C=C, NH=NH, HD=HD, TQ=512, loop_iters=1):
    assert C % 128 == 0 and T % TQ == 0 and TQ % 128 == 0
    NP = NH // 2              # head pairs
    CT = C // 128             # contraction tiles
    NTB = T // TQ             # time blocks
    TT = T // 128             # tk tiles
    NO = C // 128             # out row tiles
    QC = NH * HD
    scale = 1.0 / (HD ** 0.5)

    nc = bacc.Bacc()
    xT = nc.declare_dram_parameter("xT", [CT, NTB, 128, TQ], F32R, isOutput=False)
    wqkT = nc.declare_dram_parameter("wqkT", [C, 2 * QC], F32R, isOutput=False)
    wvT = nc.declare_dram_parameter("wvT", [CT, 128, QC], F32R, isOutput=False)
    wpT = nc.declare_dram_parameter("wpT", [NP, 128, C], F32R, isOutput=False)
    bias = nc.declare_dram_parameter("bias", [128, NO], F32, isOutput=False)
    outT = nc.declare_dram_parameter("outT", [NO, NTB, 128, TQ], F32, isOutput=True)

    with tile.TileContext(nc) as tc, ExitStack() as ctx:
        # long-lived pools first (stack allocator)
        qt_pool = ctx.enter_context(tc.tile_pool(name="qt", bufs=NP * NTB))
        kt_pool = ctx.enter_context(tc.tile_pool(name="kt", bufs=NP * NTB))
        v_pool = ctx.enter_context(tc.tile_pool(name="v", bufs=TT))
        wp_pool = ctx.enter_context(tc.tile_pool(name="wp", bufs=NP))
        bias_pool = ctx.enter_context(tc.tile_pool(name="bias", bufs=1))

        bias_sb = bias_pool.tile([128, NO], F32, tag="bias", name="bias_sb")
        nc.sync.dma_start(bias_sb[:], bias[:])
        ones_sb = bias_pool.tile([128, NH], F32, tag="ones", name="ones_sb")
        nc.gpsimd.memset(ones_sb[:], 1.0)
        wp_sb = [wp_pool.tile([128, C], F32R, tag="wp", name="wp") for p in range(NP)]
        for p in range(NP):
            nc.sync.dma_start(wp_sb[p][:], wpT[p])

        qt = {}
        kt = {}
        vt = []
        yt = {}

        def body():
            qt.clear(); kt.clear(); vt.clear(); yt.clear()
            ctx2 = ExitStack()
            st_pool = ctx2.enter_context(tc.tile_pool(name="st", bufs=2, space="PSUM"))
            o_ps_pool = ctx2.enter_context(tc.tile_pool(name="ops", bufs=4, space="PSUM"))
            pt_pool = ctx2.enter_context(tc.tile_pool(name="pt", bufs=4))
            rc_pool = ctx2.enter_context(tc.tile_pool(name="rc", bufs=4))
            osb_pool = ctx2.enter_context(tc.tile_pool(name="osb", bufs=3))

            def emit_proj_block(tb, wqk_pool, xs_pool, wvs):
                """Phase A chunk: Q^T/K^T j-tiles + V tiles for one time block."""
                xs = [xs_pool.tile([128, TQ], F32R, tag="xs", name="xs") for _ in range(CT)]
                for c in range(CT):
                    nc.sync.dma_start(xs[c][:], xT[c, tb])
                # half 0 = Q cols, half 1 = K cols of wqkT
                for half in range(2):
                    ws = []
                    for c in range(CT):
                        w = wqk_pool.tile([128, QC], F32R, tag="wqk", name="wqk")
                        nc.sync.dma_start(w[:], wqkT[128 * c:128 * (c + 1),
                                                     half * QC:(half + 1) * QC])
                        ws.append(w)
                    for jp in range(NP):
                        jt = half * NP + jp
                        ps = o_ps_pool.tile([128, TQ], F32, tag="ops", name="mm")
                        for c in range(CT):
                            nc.tensor.matmul(ps[:], ws[c][:, 128 * jp:128 * (jp + 1)], xs[c][:],
                                             start=(c == 0), stop=(c == CT - 1))
                        dst = qt_pool.tile([128, TQ], F32R, tag="qt", name="qt") if jt < NP else kt_pool.tile([128, TQ], F32R, tag="kt", name="kt")
                        nc.vector.tensor_copy(dst[:], ps[:])
                        if jt < NP:
                            qt[(jt, tb)] = dst
                        else:
                            kt[(jt - NP, tb)] = dst
                for ti in range(TQ // 128):
                    tt_i = tb * (TQ // 128) + ti
                    ps = o_ps_pool.tile([128, QC], F32, tag="ops", name="mmv")
                    for c in range(CT):
                        nc.tensor.matmul(ps[:], xs[c][:, 128 * ti:128 * (ti + 1)], wvs[c][:],
                                         start=(c == 0), stop=(c == CT - 1))
                    vtile = v_pool.tile([128, NH * (HD + 1)], F32R, tag="v", name="v")
                    v3 = vtile[:].rearrange("p (h d) -> p h d", d=HD + 1)
                    nc.vector.tensor_copy(v3[:, :, 0:HD], ps[:].rearrange("p (h d) -> p h d", d=HD))
                    nc.vector.tensor_copy(v3[:, :, HD], ones_sb[:])
                    assert len(vt) == tt_i
                    vt.append(vtile)

            def emit_attention_block(qi):
                """Phase B chunk (all pairs, one query block) + phase C for it."""
                tq0 = qi * TQ
                ntk = (tq0 + TQ) // 128
                for p in range(NP):
                    h0 = 2 * p
                    h1 = 2 * p + 1
                    o0 = o_ps_pool.tile([HD + 1, TQ], F32, tag="ops", name="ops")
                    o1 = o_ps_pool.tile([HD + 1, TQ], F32, tag="ops", name="ops2")
                    for tki in range(ntk):
                        tk0 = tki * 128
                        # diagonal narrowing: only q positions >= tk0 can
                        # attend; min width 256 (f32r matmul slows below)
                        dlt = min(max(0, tk0 - tq0), TQ - 256)
                        w = TQ - dlt
                        diag = tk0 >= tq0
                        ktile = kt[(p, tk0 // TQ)]
                        koff = tk0 % TQ
                        qtile = qt[(p, qi)]
                        st = st_pool.tile([128, 2 * TQ], F32, tag="st", name="st")
                        nc.tensor.matmul(st[:, 0:w], ktile[0:64, koff:koff + 128],
                                         qtile[0:64, dlt:TQ], start=True, stop=True)
                        nc.tensor.matmul(st[:, TQ:TQ + w], ktile[64:128, koff:koff + 128],
                                         qtile[64:128, dlt:TQ], start=True, stop=True)
                        pt = pt_pool.tile([128, 2 * TQ], F32R, tag="pt", name="pt")
                        st_v = st[:].rearrange("p (h q) -> p h q", q=TQ)[:, :, 0:w]
                        pt_v = pt[:, 0:2 * w].rearrange("p (h q) -> p h q", h=2)
                        nc.scalar.activation(pt_v, st_v, AF.Exp, scale=scale)
                        if diag:
                            bw = min(128 + (tk0 - tq0) - dlt, w)
                            base = -((tk0 - tq0) - dlt)
                            for half in range(2):
                                nc.gpsimd.affine_select(
                                    out=pt[:, half * w:half * w + bw],
                                    in_=pt[:, half * w:half * w + bw],
                                    compare_op=mybir.AluOpType.is_ge,
                                    fill=0.0, base=base,
                                    pattern=[[1, bw]], channel_multiplier=-1)
                        vtile = vt[tki]
                        v3 = vtile[:].rearrange("p (h d) -> p h d", d=HD + 1)
                        nc.tensor.matmul(o0[:, dlt:TQ], v3[:, h0, :], pt[:, 0:w],
                                         start=(tki == 0), stop=(tki == ntk - 1))
                        nc.tensor.matmul(o1[:, dlt:TQ], v3[:, h1, :], pt[:, w:2 * w],
                                         start=(tki == 0), stop=(tki == ntk - 1))
                    ytile = qt_pool.tile([128, TQ], F32R, tag="qt", name="y")
                    yt[(p, qi)] = ytile
                    for h, ops in ((0, o0), (1, o1)):
                        rc = rc_pool.tile([1, TQ], F32, tag="rc", name="rc")
                        nc.vector.reciprocal(rc[:], ops[HD:HD + 1, :])
                        bc = rc_pool.tile([HD, TQ], F32, tag="bc", name="bc")
                        nc.gpsimd.partition_broadcast(bc[:], rc[:])
                        nc.vector.tensor_mul(ytile[64 * h:64 * h + 64, :], ops[0:HD, :], bc[:])
                # phase C for this time block
                tb = qi
                for ot in range(NO):
                    ps = o_ps_pool.tile([128, TQ], F32, tag="ops", name="mmo")
                    for p in range(NP):
                        nc.tensor.matmul(ps[:], wp_sb[p][:, 128 * ot:128 * (ot + 1)], yt[(p, tb)][:],
                                         start=(p == 0), stop=(p == NP - 1))
                    osb = osb_pool.tile([128, TQ], F32, tag="osb", name="osb")
                    nc.vector.tensor_scalar_add(osb[:], ps[:], bias_sb[:, ot:ot + 1])
                    nc.sync.dma_start(outT[ot, tb], osb[:])

            # interleave: A(tb) ... B(qi=tb-1) so exp/attention overlaps the
            # next projection block on ACT/Pool/DVE while PE streams on.
            with tc.tile_pool(name="wv_s", bufs=CT) as wv_pool, \
                 tc.tile_pool(name="wqk_s", bufs=CT + 2) as wqk_pool, \
                 tc.tile_pool(name="xs", bufs=CT + 1) as xs_pool:
                wvs = []
                for c in range(CT):
                    w = wv_pool.tile([128, QC], F32R, tag="wv", name="wv")
                    nc.sync.dma_start(w[:], wvT[c])
                    wvs.append(w)
                for tb in range(NTB):
                    emit_proj_block(tb, wqk_pool, xs_pool, wvs)
                    if tb >= 1:
                        emit_attention_block(tb - 1)
            emit_attention_block(NTB - 1)
            ctx2.close()

        if loop_iters == 1:
            body()
        else:
            with tc.For_i(0, loop_iters, 1):
                body()
    nc.finalize()
    return nc


def _tile2d(a, pr, pc):
    """[R, S] -> [R//pr, S//pc, pr, pc] contiguous tiles."""
    R, S = a.shape
    return np.ascontiguousarray(
        a.reshape(R // pr, pr, S // pc, pc).transpose(0, 2, 1, 3))


def shard_inputs(x, w_attn, w_proj, b_proj, TQ=512):
    """Returns in_maps for 8 cores: core c = (b=c//2, g=c%2)."""
    CT = C // 128
    NP = NH // 2
    NTB = T // TQ
    wq, wk, wv = w_attn[0:C], w_attn[C:2 * C], w_attn[2 * C:3 * C]
    x = np.asarray(x)
    in_maps = []
    for core in range(8):
        b = core // 2
        g = core % 2
        rows = slice(g * QCOLS, (g + 1) * QCOLS)
        xTt = _tile2d(np.asarray(x[b]).T, 128, TQ)                       # [CT,NTB,128,TQ]
        wqkTt = np.ascontiguousarray(np.concatenate([wq[rows], wk[rows]], 0).T)  # [C,2QC]
        wvTt = np.ascontiguousarray(wv[rows].T.reshape(CT, 128, QCOLS))
        wpTt = np.ascontiguousarray(w_proj[:, rows].T.reshape(NP, 128, C))
        in_maps.append({
            "xT": xTt,
            "wqkT": wqkTt,
            "wvT": wvTt,
            "wpT": wpTt,
            "bias": (np.ascontiguousarray(b_proj.reshape(C // 128, 128).T)
                     if g == 0 else np.zeros((128, C // 128), np.float32)),
        })
    return in_maps


def unshard_output(outT_tiles_pair, TQ=512):
    """outT [NO,NTB,128,TQ] partials (2 cores) -> out [T, C]."""
    s = outT_tiles_pair[0] + outT_tiles_pair[1]
    NO, NTB = C // 128, T // TQ
    return s.transpose(0, 2, 1, 3).reshape(C, T).T


_NC_CACHE = {}


def kernel(x, w_attn, w_proj, b_proj):
    if "nc" not in _NC_CACHE:
        _NC_CACHE["nc"] = build()
    nc = _NC_CACHE["nc"]
    in_maps = shard_inputs(x, w_attn, w_proj, b_proj)
    res = run_bass_kernel_spmd(nc, in_maps, core_ids=list(range(8)))
    out = np.empty((B, T, C), np.float32)
    for b in range(B):
        out[b] = unshard_output([res.results[2 * b]["outT"],
                                 res.results[2 * b + 1]["outT"]])
    return out


# revision 17
# speedup vs baseline: 1.5248x; 1.4163x over previous
"""Trainium2 Bass kernel for nn_AttentionLayer (B=4, T=2048, C=1024, H=16).

Sharding (8 cores): core c = (batch b = c//2, head-group g = c%2).
Data parallel on batch, tensor parallel on heads: each core computes the
qkv projection for its 8 heads, causal flash-attention, and a partial
output projection (row split of w_proj). Host sums the two partials per
batch and re-transposes.

Per-core kernel (Bass/Tile, fp32r matmuls = TF32-like fp22 PE mode):
  phase A: qkv projection.  Q^T/K^T produced in [head_dim, t] layout
           (moving operand = x^T), V in natural [t, head_dim] layout
           (moving operand = w_v^T) with an appended ones column.
  phase B: causal attention per head-pair.  S^T = K^T.T @ Q^T row-tiled
           2 heads/matmul (contraction 64 x 2), exp on ACT (no
           max-subtract needed: logits are O(1)), causal mask via
           gpsimd.affine_select on diagonal tiles, O^T = [V|1].T @ P^T
           accumulated in PSUM; row 64 gives softmax denominators;
           normalize via reciprocal + gpsimd partition_broadcast.
  phase C: out^T = w_p^T.T @ y^T + bias (bias only on g=0 cores).

All DRAM tensors are host-pre-tiled so every DMA is one contiguous block.
"""
from contextlib import ExitStack

import numpy as np

import concourse.bacc as bacc
import concourse.mybir as mybir
import concourse.tile as tile
from concourse.bass_utils import run_bass_kernel_spmd

F32 = mybir.dt.float32
F32R = mybir.dt.float32r
AF = mybir.ActivationFunctionType

B, T, C, H = 4, 2048, 1024, 16
HD = C // H          # 64
NH = H // 2          # heads per core: 8
QCOLS = NH * HD      # 512


def build(T=T, C=C, NH=NH, HD=HD, TQ=512, loop_iters=1):
    assert C % 128 == 0 and T % TQ == 0 and TQ % 128 == 0
    NP = NH // 2              # head pairs
    CT = C // 128             # contraction tiles
    NTB = T // TQ             # time blocks
    TT = T // 128             # tk tiles
    NO = C // 128             # out row tiles
    QC = NH * HD
    scale = 1.0 / (HD ** 0.5)

    nc = bacc.Bacc()
    xT = nc.declare_dram_parameter("xT", [CT, NTB, 128, TQ], F32R, isOutput=False)
    wqkT = nc.declare_dram_parameter("wqkT", [C, 2 * QC], F32R, isOutput=False)
    wvT = nc.declare_dram_parameter("wvT", [CT, 128, QC], F32R, isOutput=False)
    wpT = nc.declare_dram_parameter("wpT", [NP, 128, C], F32R, isOutput=False)
    bias = nc.declare_dram_parameter("bias", [128, NO], F32, isOutput=False)
    outT = nc.declare_dram_parameter("outT", [NO, NTB, 128, TQ], F32, isOutput=True)

    with tile.TileContext(nc) as tc, ExitStack() as ctx:
        # long-lived pools first (stack allocator)
        qt_pool = ctx.enter_context(tc.tile_pool(name="qt", bufs=NP * NTB))
        kt_pool = ctx.enter_context(tc.tile_pool(name="kt", bufs=NP * NTB))
        v_pool = ctx.enter_context(tc.tile_pool(name="v", bufs=TT))
        wp_pool = ctx.enter_context(tc.tile_pool(name="wp", bufs=NP))
        bias_pool = ctx.enter_context(tc.tile_pool(name="bias", bufs=1))

        bias_sb = bias_pool.tile([128, NO], F32, tag="bias", name="bias_sb")
        nc.sync.dma_start(bias_sb[:], bias[:])
        ones_sb = bias_pool.tile([128, NH], F32, tag="ones", name="ones_sb")
        nc.gpsimd.memset(ones_sb[:], 1.0)
        # causal band masks: mask_a[x,y]=1 iff y>=x (used for delta<TQ-256);
        # mask_b[x,y]=1 iff y>=x+128 (used for the clamped delta=TQ-256 tile)
        ii = np.arange(128)[:, None]
        mask_a_np = (np.arange(128)[None, :] >= ii).astype(np.float32)
        mask_b_np = (np.arange(256)[None, :] >= ii + 128).astype(np.float32)
        sel_np = np.ones((1, 64), np.float32)
        mask_a_dram = nc.inline_tensor(mask_a_np, name="mask_a")
        mask_b_dram = nc.inline_tensor(mask_b_np, name="mask_b")
        sel_dram = nc.inline_tensor(sel_np, name="sel")
        mask_a = bias_pool.tile([128, 128], F32R, tag="mask_a", name="mask_a_sb")
        mask_b = bias_pool.tile([128, 256], F32R, tag="mask_b", name="mask_b_sb")
        sel_sb = bias_pool.tile([1, 64], F32R, tag="sel", name="sel_sb")
        nc.gpsimd.dma_start(mask_a[:], mask_a_dram[:])
        nc.gpsimd.dma_start(mask_b[:], mask_b_dram[:])
        nc.gpsimd.dma_start(sel_sb[:], sel_dram[:])
        wp_sb = [wp_pool.tile([128, C], F32R, tag="wp", name="wp") for p in range(NP)]
        for p in range(NP):
            nc.sync.dma_start(wp_sb[p][:], wpT[p])

        qt = {}
        kt = {}
        vt = []
        yt = {}

        def body():
            qt.clear(); kt.clear(); vt.clear(); yt.clear()
            ctx2 = ExitStack()
            st_pool = ctx2.enter_context(tc.tile_pool(name="st", bufs=2, space="PSUM"))
            o_ps_pool = ctx2.enter_context(tc.tile_pool(name="ops", bufs=4, space="PSUM"))
            pt_pool = ctx2.enter_context(tc.tile_pool(name="pt", bufs=4))
            rc_pool = ctx2.enter_context(tc.tile_pool(name="rc", bufs=4))
            osb_pool = ctx2.enter_context(tc.tile_pool(name="osb", bufs=3))

            def emit_proj_block(tb, wqk_pool, xs_pool, wvs):
                """Phase A chunk: Q^T/K^T j-tiles + V tiles for one time block."""
                xs = [xs_pool.tile([128, TQ], F32R, tag="xs", name="xs") for _ in range(CT)]
                for c in range(CT):
                    nc.sync.dma_start(xs[c][:], xT[c, tb])
                # half 0 = Q cols, half 1 = K cols of wqkT
                for half in range(2):
                    ws = []
                    for c in range(CT):
                        w = wqk_pool.tile([128, QC], F32R, tag="wqk", name="wqk")
                        nc.sync.dma_start(w[:], wqkT[128 * c:128 * (c + 1),
                                                     half * QC:(half + 1) * QC])
                        ws.append(w)
                    for jp in range(NP):
                        jt = half * NP + jp
                        ps = o_ps_pool.tile([128, TQ], F32, tag="ops", name="mm")
                        for c in range(CT):
                            nc.tensor.matmul(ps[:], ws[c][:, 128 * jp:128 * (jp + 1)], xs[c][:],
                                             start=(c == 0), stop=(c == CT - 1))
                        dst = qt_pool.tile([128, TQ], F32R, tag="qt", name="qt") if jt < NP else kt_pool.tile([128, TQ], F32R, tag="kt", name="kt")
                        nc.vector.tensor_copy(dst[:], ps[:])
                        if jt < NP:
                            qt[(jt, tb)] = dst
                        else:
                            kt[(jt - NP, tb)] = dst
                for ti in range(TQ // 128):
                    tt_i = tb * (TQ // 128) + ti
                    ps = o_ps_pool.tile([128, QC], F32, tag="ops", name="mmv")
                    for c in range(CT):
                        nc.tensor.matmul(ps[:], xs[c][:, 128 * ti:128 * (ti + 1)], wvs[c][:],
                                         start=(c == 0), stop=(c == CT - 1))
                    vtile = v_pool.tile([128, NH * (HD + 1)], F32R, tag="v", name="v")
                    v3 = vtile[:].rearrange("p (h d) -> p h d", d=HD + 1)
                    nc.vector.tensor_copy(v3[:, :, 0:HD], ps[:].rearrange("p (h d) -> p h d", d=HD))
                    nc.vector.tensor_copy(v3[:, :, HD], ones_sb[:])
                    assert len(vt) == tt_i
                    vt.append(vtile)

            def emit_attention_block(qi):
                """Phase B chunk (all pairs, one query block) + phase C for it."""
                tq0 = qi * TQ
                ntk = (tq0 + TQ) // 128
                for p in range(NP):
                    h0 = 2 * p
                    h1 = 2 * p + 1
                    o0 = o_ps_pool.tile([HD + 1, TQ], F32, tag="ops", name="ops")
                    o1 = o_ps_pool.tile([HD + 1, TQ], F32, tag="ops", name="ops2")
                    for tki in range(ntk):
                        tk0 = tki * 128
                        # diagonal narrowing: only q positions >= tk0 can
                        # attend; min width 256 (f32r matmul slows below)
                        dlt = min(max(0, tk0 - tq0), TQ - 256)
                        w = TQ - dlt
                        diag = tk0 >= tq0
                        ktile = kt[(p, tk0 // TQ)]
                        koff = tk0 % TQ
                        qtile = qt[(p, qi)]
                        st = st_pool.tile([128, 2 * TQ], F32, tag="st", name="st")
                        nc.tensor.matmul(st[:, 0:w], ktile[0:64, koff:koff + 128],
                                         qtile[0:64, dlt:TQ], start=True, stop=True)
                        nc.tensor.matmul(st[:, TQ:TQ + w], ktile[64:128, koff:koff + 128],
                                         qtile[64:128, dlt:TQ], start=True, stop=True)
                        pt = pt_pool.tile([128, 2 * TQ], F32R, tag="pt", name="pt")
                        st_v = st[:].rearrange("p (h q) -> p h q", q=TQ)[:, :, 0:w]
                        pt_v = pt[:, 0:2 * w].rearrange("p (h q) -> p h q", h=2)
                        nc.scalar.activation(pt_v, st_v, AF.Exp, scale=scale)
                        if diag:
                            clamped = (tk0 - tq0) > dlt
                            m = mask_b if clamped else mask_a
                            bw = 256 if clamped else 128
                            for half in range(2):
                                nc.vector.tensor_mul(pt[:, half * w:half * w + bw],
                                                     pt[:, half * w:half * w + bw],
                                                     m[:, 0:bw])
                        vtile = vt[tki]
                        v3 = vtile[:].rearrange("p (h d) -> p h d", d=HD + 1)
                        nc.tensor.matmul(o0[:, dlt:TQ], v3[:, h0, :], pt[:, 0:w],
                                         start=(tki == 0), stop=(tki == ntk - 1))
                        nc.tensor.matmul(o1[:, dlt:TQ], v3[:, h1, :], pt[:, w:2 * w],
                                         start=(tki == 0), stop=(tki == ntk - 1))
                    ytile = qt_pool.tile([128, TQ], F32R, tag="qt", name="y")
                    yt[(p, qi)] = ytile
                    rcA = rc_pool.tile([1, TQ], F32R, tag="rc", name="rcA")
                    rcB = rc_pool.tile([1, TQ], F32R, tag="rcb", name="rcB")
                    with nc.allow_low_precision(reason="f32r==fp32 bits; denominators kept full fp32"):
                        nc.vector.reciprocal(rcA[:], o0[HD:HD + 1, :])
                        nc.vector.reciprocal(rcB[:], o1[HD:HD + 1, :])
                    bc0 = o_ps_pool.tile([HD, TQ], F32, tag="ops", name="bc0")
                    bc1 = o_ps_pool.tile([HD, TQ], F32, tag="ops", name="bc1")
                    nc.tensor.matmul(bc0[:], sel_sb[:], rcA[:], start=True, stop=True)
                    nc.tensor.matmul(bc1[:], sel_sb[:], rcB[:], start=True, stop=True)
                    nc.scalar.activation(ytile[0:64, :], o0[0:HD, :], AF.Copy)
                    nc.scalar.activation(ytile[64:128, :], o1[0:HD, :], AF.Copy)
                    nc.vector.tensor_mul(ytile[0:64, :], ytile[0:64, :], bc0[:])
                    nc.vector.tensor_mul(ytile[64:128, :], ytile[64:128, :], bc1[:])
                # phase C for this time block
                tb = qi
                for ot in range(NO):
                    ps = o_ps_pool.tile([128, TQ], F32, tag="ops", name="mmo")
                    for p in range(NP):
                        nc.tensor.matmul(ps[:], wp_sb[p][:, 128 * ot:128 * (ot + 1)], yt[(p, tb)][:],
                                         start=(p == 0), stop=(p == NP - 1))
                    osb = osb_pool.tile([128, TQ], F32, tag="osb", name="osb")
                    nc.vector.tensor_scalar_add(osb[:], ps[:], bias_sb[:, ot:ot + 1])
                    nc.sync.dma_start(outT[ot, tb], osb[:])

            # interleave: A(tb) ... B(qi=tb-1) so exp/attention overlaps the
            # next projection block on ACT/Pool/DVE while PE streams on.
            with tc.tile_pool(name="wv_s", bufs=CT) as wv_pool, \
                 tc.tile_pool(name="wqk_s", bufs=CT + 2) as wqk_pool, \
                 tc.tile_pool(name="xs", bufs=CT + 1) as xs_pool:
                wvs = []
                for c in range(CT):
                    w = wv_pool.tile([128, QC], F32R, tag="wv", name="wv")
                    nc.sync.dma_start(w[:], wvT[c])
                    wvs.append(w)
                for tb in range(NTB):
                    emit_proj_block(tb, wqk_pool, xs_pool, wvs)
                    if tb >= 1:
                        emit_attention_block(tb - 1)
            emit_attention_block(NTB - 1)
            ctx2.close()

        if loop_iters == 1:
            body()
        else:
            with tc.For_i(0, loop_iters, 1):
                body()
    nc.finalize()
    return nc


def _tile2d(a, pr, pc):
    """[R, S] -> [R//pr, S//pc, pr, pc] contiguous tiles."""
    R, S = a.shape
    return np.ascontiguousarray(
        a.reshape(R // pr, pr, S // pc, pc).transpose(0, 2, 1, 3))


def shard_inputs(x, w_attn, w_proj, b_proj, TQ=512):
    """Returns in_maps for 8 cores: core c = (b=c//2, g=c%2)."""
    CT = C // 128
    NP = NH // 2
    NTB = T // TQ
    wq, wk, wv = w_attn[0:C], w_attn[C:2 * C], w_attn[2 * C:3 * C]
    x = np.asarray(x)
    in_maps = []
    for core in range(8):
        b = core // 2
        g = core % 2
        rows = slice(g * QCOLS, (g + 1) * QCOLS)
        xTt = _tile2d(np.asarray(x[b]).T, 128, TQ)                       # [CT,NTB,128,TQ]
        wqkTt = np.ascontiguousarray(np.concatenate([wq[rows], wk[rows]], 0).T)  # [C,2QC]
        wvTt = np.ascontiguousarray(wv[rows].T.reshape(CT, 128, QCOLS))
        wpTt = np.ascontiguousarray(w_proj[:, rows].T.reshape(NP, 128, C))
        in_maps.append({
            "xT": xTt,
            "wqkT": wqkTt,
            "wvT": wvTt,
            "wpT": wpTt,
            "bias": (np.ascontiguousarray(b_proj.reshape(C // 128, 128).T)
                     if g == 0 else np.zeros((128, C // 128), np.float32)),
        })
    return in_maps


def unshard_output(outT_tiles_pair, TQ=512):
    """outT [NO,NTB,128,TQ] partials (2 cores) -> out [T, C]."""
    s = outT_tiles_pair[0] + outT_tiles_pair[1]
    NO, NTB = C // 128, T // TQ
    return s.transpose(0, 2, 1, 3).reshape(C, T).T


_NC_CACHE = {}


def kernel(x, w_attn, w_proj, b_proj):
    if "nc" not in _NC_CACHE:
        _NC_CACHE["nc"] = build()
    nc = _NC_CACHE["nc"]
    in_maps = shard_inputs(x, w_attn, w_proj, b_proj)
    res = run_bass_kernel_spmd(nc, in_maps, core_ids=list(range(8)))
    out = np.empty((B, T, C), np.float32)
    for b in range(B):
        out[b] = unshard_output([res.results[2 * b]["outT"],
                                 res.results[2 * b + 1]["outT"]])
    return out
